# revision 21
# baseline (speedup 1.0000x reference)
"""Grouped linear (MoE routing) Trainium2 kernel.

y[t] = x[t] @ weight[g_t] + bias[g_t],  g_t = group_indices[t]

Data-parallel over 8 cores (8192 tokens each), weights replicated.

Routing is resolved on the host while sharding: each core's tokens are
stable-sorted by group and x is laid out contraction-major as
xt[din, slot] with group blocks padded to 128-slot tiles (pad columns
zero).  The device kernel is then a pure streaming grouped GEMM:

  1. Sequential HWDGE loads of 512-slot xt chunks (4-chunk prefetch)
     give lhsT tiles [128 din, 128 slots] with no on-chip transpose or
     gather.
  2. Per 128-slot tile, 8 K-chunks x 2 N-chunks of (K=128, M=128,
     N=512) bf16 matmuls accumulate f32 in PSUM (all 8 banks in
     flight); group weights stream through SBUF double-buffered.
  3. DVE fuses bias add (pre-broadcast per group, bf16) with
     PSUM->SBUF copy; indirect_dma_start scatters each tile's 128 rows
     to out[token] using host-computed slot->token offsets (pads
     skipped via bounds_check).  Scatters round-robin across NOUT
     separate output tensors: consecutive scatters to one tensor are
     WAW-chained (desc-gen + ~2us completion latency each), so
     interleaving NOUT independent chains keeps the per-tile scatter
     pace under the PE's per-tile compute time.  (Batching >128 rows
     into one scatter via a multi-column offset AP wedges the SWDGE
     exec unit - NRT_EXEC_UNIT_UNRECOVERABLE - so more tensors, not
     bigger scatters.)  The host merges the NOUT shards row-wise while
     unsharding, using the tile->token map it computed for routing.

Output is bf16 (the reference itself accumulates in bf16); the host
upcasts to f32 while unsharding.  Back-pressure keeps the PE
continuously fed so the HAM clock stays at 2.4 GHz.
"""

import sys

import numpy as np

sys.path.insert(0, "/opt/trn_rl_repo")

from concourse import bacc, bass, mybir, tile  # noqa: E402

N_CORES = 8
BATCH = 65536
TOK = BATCH // N_CORES  # tokens per core
DIN = 1024
DOUT = 1024
NG = 8
P = 128

FP32 = mybir.dt.float32
BF16 = mybir.dt.bfloat16
I32 = mybir.dt.int32

SENTINEL = 99999  # > TOK-1: skipped by bounds_check on output scatter

Alu = mybir.AluOpType

XCH = 512  # slots per xt load chunk (1 KB per descriptor)
NOUT = 8  # independent output tensors (parallel scatter WAW chains)


def build_kernel(cap):
    """cap[g] = static slot capacity of group g (multiple of 128, >=
    per-core count of group g on every core)."""
    cap = [int(c) for c in cap]
    assert all(c % P == 0 for c in cap) and sum(cap) % P == 0
    nslots = sum(cap)
    ntiles = nslots // P

    tile_group = []
    for g in range(NG):
        tile_group += [g] * (cap[g] // P)

    nc = bacc.Bacc(
        "TRN2",
        target_bir_lowering=False,
        debug=False,
        num_devices=N_CORES,
    )

    xt_d = nc.dram_tensor("xt", [DIN, nslots], BF16, kind="ExternalInput").ap()
    w_d = nc.dram_tensor("w", [NG, DIN, DOUT], BF16, kind="ExternalInput").ap()
    b_d = nc.dram_tensor("b", [NG, DOUT], BF16, kind="ExternalInput").ap()
    yo_d = nc.dram_tensor("yo", [P, ntiles], I32, kind="ExternalInput").ap()
    out_d = [
        nc.dram_tensor(f"out{k}", [TOK, DOUT], BF16, kind="ExternalOutput").ap()
        for k in range(NOUT)
    ]

    n_xch = (nslots + XCH - 1) // XCH
    xch_n = [min(XCH, nslots - i * XCH) for i in range(n_xch)]

    with tile.TileContext(nc) as tc:
        with (
            tc.tile_pool(name="sbuf", bufs=1) as sb,
            tc.tile_pool(name="wpool", bufs=2) as wpool,
            tc.tile_pool(name="xpool", bufs=5) as xpool,
            tc.tile_pool(name="ypool", bufs=8) as ypool,
            tc.tile_pool(name="psum", bufs=8, space="PSUM") as psum,
        ):
            yo_sb = sb.tile([P, ntiles], I32, tag="yo")
            nc.sync.dma_start(out=yo_sb[:], in_=yo_d[:])

            xt_r = xt_d.rearrange("(c p) s -> p c s", p=P)

            def load_x(ch):
                n = xch_n[ch]
                xtile = xpool.tile([P, DIN // P, n], BF16, tag="x")
                nc.sync.dma_start(
                    out=xtile[:], in_=xt_r[:, :, ch * XCH : ch * XCH + n]
                )
                return xtile

            def load_w(g, split=False):
                # scalar engine: separate HW queue, overlaps the sync-issued
                # x loads (the head is otherwise serialized on one queue)
                wt = wpool.tile([P, DIN // P, DOUT], BF16, tag="w")
                w_r = w_d[g].rearrange("(c p) j -> p c j", p=P)
                if split:
                    # halves on both HWDGE queues so w0 lands with x0
                    nc.scalar.dma_start(out=wt[:, 0:4, :], in_=w_r[:, 0:4, :])
                    nc.sync.dma_start(out=wt[:, 4:8, :], in_=w_r[:, 4:8, :])
                else:
                    nc.scalar.dma_start(out=wt[:], in_=w_r)
                return wt

            # first x chunk and first weight before everything else
            xtiles = {0: load_x(0)}
            w_sb = {0: load_w(0, split=True)}

            # ---------------- bias broadcast (bf16) ----------------
            bias_rep = sb.tile([P, NG, DOUT], BF16, tag="bias_rep")
            for g in range(NG):
                b16 = sb.tile([1, DOUT], BF16, tag="b16")
                nc.sync.dma_start(out=b16[:], in_=b_d[g : g + 1, :])
                nc.gpsimd.partition_broadcast(bias_rep[:, g, :], b16[:])

            for g in range(1, NG):
                w_sb[g] = load_w(g)

            PREF = 4
            for ch in range(1, min(PREF, n_xch)):
                xtiles[ch] = load_x(ch)

            # ---------------- streaming grouped GEMM ----------------
            t = 0
            for ch in range(n_xch):
                if ch + PREF < n_xch:
                    xtiles[ch + PREF] = load_x(ch + PREF)
                xtile = xtiles.pop(ch)
                for off in range(0, xch_n[ch], P):
                    g = tile_group[t]
                    y_st = ypool.tile([P, DOUT], BF16, tag="y")
                    ps0 = psum.tile([P, 512], FP32, tag="acc")
                    ps1 = psum.tile([P, 512], FP32, tag="acc")
                    acc = [ps0, ps1]
                    # ic outer: both N-halves reuse the same stationary lhsT
                    for ic in range(DIN // P):
                        for jc in range(2):
                            nc.tensor.matmul(
                                out=acc[jc][:],
                                lhsT=xtile[:, ic, off : off + P],
                                rhs=w_sb[g][:, ic, jc * 512 : (jc + 1) * 512],
                                start=(ic == 0),
                                stop=(ic == DIN // P - 1),
                            )
                    for jc in range(2):
                        nc.vector.tensor_tensor(
                            out=y_st[:, jc * 512 : (jc + 1) * 512],
                            in0=acc[jc][:],
                            in1=bias_rep[:, g, jc * 512 : (jc + 1) * 512],
                            op=Alu.add,
                        )
                    nc.gpsimd.indirect_dma_start(
                        out=out_d[t % NOUT][:],
                        out_offset=bass.IndirectOffsetOnAxis(
                            ap=yo_sb[:, t : t + 1], axis=0
                        ),
                        in_=y_st[:],
                        in_offset=None,
                        bounds_check=TOK - 1,
                        oob_is_err=False,
                    )
                    t += 1
            assert t == ntiles

    nc.compile()
    return nc


def _plan_caps(gi: np.ndarray) -> np.ndarray:
    counts = np.zeros((N_CORES, NG), dtype=np.int64)
    for c in range(N_CORES):
        counts[c] = np.bincount(gi[c * TOK : (c + 1) * TOK], minlength=NG)
    mx = counts.max(axis=0)
    return ((mx + P - 1) // P) * P


def _route_core(x_c, gi_c, cap):
    """Sort one core's tokens by group into padded 128-slot blocks.

    Returns xt [DIN, nslots] bf16 (contraction-major, pads zero),
    yoff [P, ntiles] int32 (slot -> token, pads SENTINEL), and
    owner [TOK] (which of the NOUT output tensors holds each token)."""
    nslots = int(cap.sum())
    order = np.argsort(gi_c, kind="stable")
    counts = np.bincount(gi_c, minlength=NG)
    gbase = np.concatenate(([0], np.cumsum(cap)))[:NG]
    cstart = np.concatenate(([0], np.cumsum(counts)))[:NG]

    slot_token = np.full(nslots, -1, dtype=np.int64)
    xt = np.zeros((DIN, nslots), dtype=x_c.dtype)
    for g in range(NG):
        n = int(counts[g])
        toks = order[cstart[g] : cstart[g] + n]
        slot_token[gbase[g] : gbase[g] + n] = toks
        xt[:, gbase[g] : gbase[g] + n] = x_c[toks].T

    yoff = np.where(slot_token >= 0, slot_token, SENTINEL)
    yoff = np.ascontiguousarray(yoff.reshape(-1, P).T).astype(np.int32)

    real = slot_token >= 0
    owner = np.empty(TOK, dtype=np.int64)
    owner[slot_token[real]] = (np.arange(nslots) // P)[real] % NOUT
    return np.ascontiguousarray(xt), yoff, owner


LAST_RESULTS = None  # stashed BassKernelResults for external profiling


def kernel(x, weight, bias, group_indices):
    global LAST_RESULTS
    from concourse.bass_utils import run_bass_kernel_spmd

    x = np.asarray(x)
    weight = np.asarray(weight)
    bias = np.asarray(bias)
    gi = np.ascontiguousarray(np.asarray(group_indices, dtype=np.int32))

    cap = _plan_caps(gi)
    nc = build_kernel(cap)

    in_maps = []
    owners = []
    for c in range(N_CORES):
        xt, yoff, owner = _route_core(
            np.ascontiguousarray(x[c * TOK : (c + 1) * TOK]),
            gi[c * TOK : (c + 1) * TOK],
            cap,
        )
        in_maps.append({"xt": xt, "w": weight, "b": bias, "yo": yoff})
        owners.append(owner)
    res = run_bass_kernel_spmd(nc, in_maps, core_ids=list(range(N_CORES)))
    LAST_RESULTS = res

    out = np.empty((BATCH, DOUT), dtype=np.float32)
    for c in range(N_CORES):
        out_c = out[c * TOK : (c + 1) * TOK]
        for k in range(NOUT):
            m = owners[c] == k
            out_c[m] = res.results[c][f"out{k}"][m].astype(np.float32)
    return out


# revision 24
# speedup vs baseline: 1.0056x; 1.0056x over previous
"""Grouped linear (MoE routing) Trainium2 kernel.

y[t] = x[t] @ weight[g_t] + bias[g_t],  g_t = group_indices[t]

Data-parallel over 8 cores (8192 tokens each), weights replicated.

Routing is resolved on the host while sharding: each core's tokens are
stable-sorted by group and x is laid out contraction-major as
xt[din, slot] with group blocks padded to 128-slot tiles (pad columns
zero).  The device kernel is then a pure streaming grouped GEMM:

  1. Sequential HWDGE loads of 512-slot xt chunks (4-chunk prefetch)
     give lhsT tiles [128 din, 128 slots] with no on-chip transpose or
     gather.
  2. Per 128-slot tile, 8 K-chunks x 2 N-chunks of (K=128, M=128,
     N=512) bf16 matmuls accumulate f32 in PSUM (all 8 banks in
     flight); group weights stream through SBUF double-buffered.
  3. DVE fuses bias add (pre-broadcast per group, bf16) with
     PSUM->SBUF copy; indirect_dma_start scatters each tile's 128 rows
     to out[token] using host-computed slot->token offsets (pads
     skipped via bounds_check).  Scatters round-robin across NOUT
     separate output tensors: consecutive scatters to one tensor are
     WAW-chained (desc-gen + ~2us completion latency each), so
     interleaving NOUT independent chains keeps the per-tile scatter
     pace under the PE's per-tile compute time.  (Batching >128 rows
     into one scatter via a multi-column offset AP wedges the SWDGE
     exec unit - NRT_EXEC_UNIT_UNRECOVERABLE - so more tensors, not
     bigger scatters.)  The host merges the NOUT shards row-wise while
     unsharding, using the tile->token map it computed for routing.

Output is bf16 (the reference itself accumulates in bf16); the host
upcasts to f32 while unsharding.  Back-pressure keeps the PE
continuously fed so the HAM clock stays at 2.4 GHz.
"""

import sys

import numpy as np

sys.path.insert(0, "/opt/trn_rl_repo")

from concourse import bacc, bass, mybir, tile  # noqa: E402

N_CORES = 8
BATCH = 65536
TOK = BATCH // N_CORES  # tokens per core
DIN = 1024
DOUT = 1024
NG = 8
P = 128

FP32 = mybir.dt.float32
BF16 = mybir.dt.bfloat16
I32 = mybir.dt.int32

SENTINEL = 99999  # > TOK-1: skipped by bounds_check on output scatter

Alu = mybir.AluOpType

XCH = 512  # slots per xt load chunk (1 KB per descriptor)
NOUT = 8  # independent output tensors (parallel scatter WAW chains)


def build_kernel(cap):
    """cap[g] = static slot capacity of group g (multiple of 128, >=
    per-core count of group g on every core)."""
    cap = [int(c) for c in cap]
    assert all(c % P == 0 for c in cap) and sum(cap) % P == 0
    nslots = sum(cap)
    ntiles = nslots // P

    tile_group = []
    for g in range(NG):
        tile_group += [g] * (cap[g] // P)

    nc = bacc.Bacc(
        "TRN2",
        target_bir_lowering=False,
        debug=False,
        num_devices=N_CORES,
    )

    xt_d = nc.dram_tensor("xt", [DIN, nslots], BF16, kind="ExternalInput").ap()
    w_d = nc.dram_tensor("w", [NG, DIN, DOUT], BF16, kind="ExternalInput").ap()
    b_d = nc.dram_tensor("b", [NG, DOUT], BF16, kind="ExternalInput").ap()
    yo_d = nc.dram_tensor("yo", [P, ntiles], I32, kind="ExternalInput").ap()
    out_d = [
        nc.dram_tensor(f"out{k}", [TOK, DOUT], BF16, kind="ExternalOutput").ap()
        for k in range(NOUT)
    ]

    # small leading chunks so the first tiles' data lands quickly; the
    # head is bound by x-chunk-0 + w0 landing through shared SDMA engines
    xch_n = []
    left = nslots
    for want in (P, P, 2 * P):
        n = min(want, left)
        if n:
            xch_n.append(n)
            left -= n
    while left:
        n = min(XCH, left)
        xch_n.append(n)
        left -= n
    xch_off = [0]
    for n in xch_n:
        xch_off.append(xch_off[-1] + n)
    n_xch = len(xch_n)

    with tile.TileContext(nc) as tc:
        with (
            tc.tile_pool(name="sbuf", bufs=1) as sb,
            tc.tile_pool(name="wpool", bufs=2) as wpool,
            tc.tile_pool(name="xpool", bufs=5) as xpool,
            tc.tile_pool(name="ypool", bufs=8) as ypool,
            tc.tile_pool(name="psum", bufs=8, space="PSUM") as psum,
        ):
            yo_sb = sb.tile([P, ntiles], I32, tag="yo")
            nc.sync.dma_start(out=yo_sb[:], in_=yo_d[:])

            xt_r = xt_d.rearrange("(c p) s -> p c s", p=P)

            def load_x(ch):
                n = xch_n[ch]
                s0 = xch_off[ch]
                xtile = xpool.tile([P, DIN // P, n], BF16, tag="x")
                nc.sync.dma_start(out=xtile[:], in_=xt_r[:, :, s0 : s0 + n])
                return xtile

            def load_w(g, split=False):
                # scalar engine: separate HW queue, overlaps the sync-issued
                # x loads (the head is otherwise serialized on one queue)
                wt = wpool.tile([P, DIN // P, DOUT], BF16, tag="w")
                w_r = w_d[g].rearrange("(c p) j -> p c j", p=P)
                if split:
                    # halves on both HWDGE queues so w0 lands with x0
                    nc.scalar.dma_start(out=wt[:, 0:4, :], in_=w_r[:, 0:4, :])
                    nc.sync.dma_start(out=wt[:, 4:8, :], in_=w_r[:, 4:8, :])
                else:
                    nc.scalar.dma_start(out=wt[:], in_=w_r)
                return wt

            # first x chunk and first weight before everything else
            xtiles = {0: load_x(0)}
            w_sb = {0: load_w(0, split=True)}

            # ---------------- bias broadcast (bf16) ----------------
            # one contiguous load: 8 tiny single-partition loads each stall
            # the issuing engine ~5us
            ball = sb.tile([1, NG * DOUT], BF16, tag="ball")
            nc.sync.dma_start(out=ball[:], in_=b_d.rearrange("g j -> (g j)")[None, :])
            bias_rep = sb.tile([P, NG, DOUT], BF16, tag="bias_rep")
            for g in range(NG):
                nc.gpsimd.partition_broadcast(
                    bias_rep[:, g, :], ball[:, g * DOUT : (g + 1) * DOUT]
                )

            for g in range(1, NG):
                w_sb[g] = load_w(g)

            PREF = 4
            for ch in range(1, min(PREF, n_xch)):
                xtiles[ch] = load_x(ch)

            # ---------------- streaming grouped GEMM ----------------
            t = 0
            for ch in range(n_xch):
                if ch + PREF < n_xch:
                    xtiles[ch + PREF] = load_x(ch + PREF)
                xtile = xtiles.pop(ch)
                for off in range(0, xch_n[ch], P):
                    g = tile_group[t]
                    y_st = ypool.tile([P, DOUT], BF16, tag="y")
                    ps0 = psum.tile([P, 512], FP32, tag="acc")
                    ps1 = psum.tile([P, 512], FP32, tag="acc")
                    acc = [ps0, ps1]
                    # ic outer: both N-halves reuse the same stationary lhsT
                    for ic in range(DIN // P):
                        for jc in range(2):
                            nc.tensor.matmul(
                                out=acc[jc][:],
                                lhsT=xtile[:, ic, off : off + P],
                                rhs=w_sb[g][:, ic, jc * 512 : (jc + 1) * 512],
                                start=(ic == 0),
                                stop=(ic == DIN // P - 1),
                            )
                    for jc in range(2):
                        nc.vector.tensor_tensor(
                            out=y_st[:, jc * 512 : (jc + 1) * 512],
                            in0=acc[jc][:],
                            in1=bias_rep[:, g, jc * 512 : (jc + 1) * 512],
                            op=Alu.add,
                        )
                    nc.gpsimd.indirect_dma_start(
                        out=out_d[t % NOUT][:],
                        out_offset=bass.IndirectOffsetOnAxis(
                            ap=yo_sb[:, t : t + 1], axis=0
                        ),
                        in_=y_st[:],
                        in_offset=None,
                        bounds_check=TOK - 1,
                        oob_is_err=False,
                    )
                    t += 1
            assert t == ntiles

    nc.compile()
    return nc


def _plan_caps(gi: np.ndarray) -> np.ndarray:
    counts = np.zeros((N_CORES, NG), dtype=np.int64)
    for c in range(N_CORES):
        counts[c] = np.bincount(gi[c * TOK : (c + 1) * TOK], minlength=NG)
    mx = counts.max(axis=0)
    return ((mx + P - 1) // P) * P


def _route_core(x_c, gi_c, cap):
    """Sort one core's tokens by group into padded 128-slot blocks.

    Returns xt [DIN, nslots] bf16 (contraction-major, pads zero),
    yoff [P, ntiles] int32 (slot -> token, pads SENTINEL), and
    owner [TOK] (which of the NOUT output tensors holds each token)."""
    nslots = int(cap.sum())
    order = np.argsort(gi_c, kind="stable")
    counts = np.bincount(gi_c, minlength=NG)
    gbase = np.concatenate(([0], np.cumsum(cap)))[:NG]
    cstart = np.concatenate(([0], np.cumsum(counts)))[:NG]

    slot_token = np.full(nslots, -1, dtype=np.int64)
    xt = np.zeros((DIN, nslots), dtype=x_c.dtype)
    for g in range(NG):
        n = int(counts[g])
        toks = order[cstart[g] : cstart[g] + n]
        slot_token[gbase[g] : gbase[g] + n] = toks
        xt[:, gbase[g] : gbase[g] + n] = x_c[toks].T

    yoff = np.where(slot_token >= 0, slot_token, SENTINEL)
    yoff = np.ascontiguousarray(yoff.reshape(-1, P).T).astype(np.int32)

    real = slot_token >= 0
    owner = np.empty(TOK, dtype=np.int64)
    owner[slot_token[real]] = (np.arange(nslots) // P)[real] % NOUT
    return np.ascontiguousarray(xt), yoff, owner


LAST_RESULTS = None  # stashed BassKernelResults for external profiling


def kernel(x, weight, bias, group_indices):
    global LAST_RESULTS
    from concourse.bass_utils import run_bass_kernel_spmd

    x = np.asarray(x)
    weight = np.asarray(weight)
    bias = np.asarray(bias)
    gi = np.ascontiguousarray(np.asarray(group_indices, dtype=np.int32))

    cap = _plan_caps(gi)
    nc = build_kernel(cap)

    in_maps = []
    owners = []
    for c in range(N_CORES):
        xt, yoff, owner = _route_core(
            np.ascontiguousarray(x[c * TOK : (c + 1) * TOK]),
            gi[c * TOK : (c + 1) * TOK],
            cap,
        )
        in_maps.append({"xt": xt, "w": weight, "b": bias, "yo": yoff})
        owners.append(owner)
    res = run_bass_kernel_spmd(nc, in_maps, core_ids=list(range(N_CORES)))
    LAST_RESULTS = res

    out = np.empty((BATCH, DOUT), dtype=np.float32)
    for c in range(N_CORES):
        out_c = out[c * TOK : (c + 1) * TOK]
        for k in range(NOUT):
            m = owners[c] == k
            out_c[m] = res.results[c][f"out{k}"][m].astype(np.float32)
    return out
